# revision 51
# baseline (speedup 1.0000x reference)
"""Trainium2 Bass kernel for nn_Attention_62861141344964.

Full-input contract: kernel(**inputs) takes the unsharded inputs and returns
the full-shape output. Internally shards across 8 NeuronCores as
(batch, head-pair): core c handles batch c//4 and heads {2*(c%4), 2*(c%4)+1}.

Per-core pipeline (ACT-exp is the bottleneck engine; everything else is
arranged around keeping its exp stream dense):
  - prologue per n-block: x DMA -> xsq(fp8, DVE) -> sumsq (fp8 DoubleRow
    matmul) -> DMA round-trip to transposed layout -> rsqrt via DVE bit-trick
    + 2 Newton steps (no ACT sqrt, so ACT runs exp only, one table load) ->
    broadcast (Pool) -> q/k projections (f32r) -> q2/k2 scaled (DVE).
  - attention: sim matmuls f32r -> exp on ACT with bias -2 emitting fp8e4
    directly -> AV as fp8 DoubleRow over j-tile pairs (0.5 cyc/row, two
    j-tiles per instruction) with plain-fp8 orphans; denominator rides row 0
    of vT/av (ones trick).
  - tail per i-block: reciprocal(den) -> Pool partition_broadcast -> DVE
    normalize into `on` (fp8) -> output projection as one fp8-DoubleRow
    matmul per 128-chunk (both heads contracted together) -> residual add on
    Pool -> DMA out.
The host folds g*sqrt(c) (and q's 1/8) into the weights and sums the 4
partial outputs per batch.
"""

import sys

sys.path.insert(0, "/opt/trn_rl_repo")

import numpy as np

HEADS = 8
DH = 64
DIM = 512
B = 2
HWS = 48
N = HWS * HWS  # 2304
KT = 4  # k-tiles of 128 over DIM
JT = 18  # j-tiles of 128 over N
# prologue (n-block) order: small block first for a short lead-in chain;
# attention (i-block) order: 512-blocks first, 256 last (small tail)
NBLKS = [(2048, 256), (1536, 512), (1024, 512), (512, 512), (0, 512)]
IBLKS = [(1536, 512), (1024, 512), (512, 512), (0, 512), (2048, 128), (2176, 128)]
BLK_OF_COL = {2048: 0, 2176: 0, 1536: 1, 1024: 2, 512: 3, 0: 4}
MAGIC_SQRT = 0x1FBD1DF5
SQRT32 = 5.656854249492381

_CACHE = {}


def _build_program(debug=False):
    import concourse.bass as bass  # noqa: F401
    import concourse.mybir as mybir
    import concourse.tile as tile
    from concourse import bacc

    f32 = mybir.dt.float32
    f32r = mybir.dt.float32r
    bf16 = mybir.dt.bfloat16
    f8 = mybir.dt.float8e4
    i32 = mybir.dt.int32
    AF = mybir.ActivationFunctionType
    OP = mybir.AluOpType
    DRM = mybir.MatmulPerfMode.DoubleRow

    nc = bacc.Bacc("TRN2", target_bir_lowering=False, debug=False, num_devices=8)

    xb_d = nc.dram_tensor("xbin", [DIM, N], bf16, kind="ExternalInput").ap()
    wqk_d = nc.dram_tensor("wqk", [DIM, 256], bf16, kind="ExternalInput").ap()
    wv_d = nc.dram_tensor("wv", [DIM, 128], bf16, kind="ExternalInput").ap()
    wp_d = nc.dram_tensor("wp", [65, 2, DIM], f8, kind="ExternalInput").ap()
    y_d = nc.dram_tensor("y", [DIM, N], f32, kind="ExternalOutput").ap()
    dbg = {}
    if debug:
        for nm, shp, dt in [("q2", [128, N], f32), ("k2", [128, N], f32),
                            ("vT", [128, JT, 144], f32), ("on", [65, 2, N], f32),
                            ("s_bc", [128, N], f32), ("s_colT", [128, JT], f32)]:
            dbg[nm] = nc.dram_tensor("dbg_" + nm, shp, dt, kind="ExternalOutput").ap()

    with tile.TileContext(nc) as tc:
        big = tc.alloc_tile_pool(name="big", bufs=1)
        work = tc.alloc_tile_pool(name="work", bufs=2)
        rsq = tc.alloc_tile_pool(name="rsq", bufs=6)
        pg = tc.alloc_tile_pool(name="pg", bufs=1, space="PSUM")
        pav = tc.alloc_tile_pool(name="pav", bufs=4, space="PSUM")

        # ---------- persistent tiles ----------
        xb = big.tile([128, KT, N], bf16)
        q2 = big.tile([128, N], bf16)
        k2 = big.tile([128, N], bf16)
        s_bc = big.tile([128, N], f32)
        s_row = big.tile([1, N], f32)
        t_colT = big.tile([128, JT], f32)
        s_colT = big.tile([128, JT], f32)
        vT = big.tile([128, JT, 144], f8)
        on = big.tile([65, 2, N], f8)
        wqk_s = big.tile([128, KT, 256], bf16)
        wv_s = big.tile([128, KT, 128], bf16)
        wp_s = big.tile([65, 2, DIM], f8)
        ones8 = big.tile([128, 2, 16], f8)
        ones_bf = big.tile([128, 16], bf16)
        r_rows = big.tile([1, 5, 512], f32)
        r_chunks = big.tile([128, 5, 4], f32)
        bias_m2 = big.tile([128, 1], f32)
        e64 = big.tile([128, 128], f32r)  # row 64 = ones: PE partition-bcast of row 64
        e0 = big.tile([128, 128], f32r)   # row 0 = ones
        den_pad = [big.tile([128, 512], f32r, name="den_pad0"),
                   big.tile([128, 512], f32r, name="den_pad1")]
        s_pad = [big.tile([128, 512], f32r, name="s_pad0"),
                 big.tile([128, 512], f32r, name="s_pad1")]


        nc.gpsimd.memset(ones8[:], 1.0)
        nc.gpsimd.memset(ones_bf[:], 1.0)
        nc.vector.memset(bias_m2[:], -2.0)
        nc.gpsimd.memset(vT[:, :, 64:65], 1.0)
        nc.gpsimd.memset(vT[:, :, 136:137], 1.0)
        nc.vector.memset(e64[:].bitcast(f32), 0.0)
        nc.vector.memset(e64[64:65, :].bitcast(f32), 1.0)
        nc.vector.memset(e0[:].bitcast(f32), 0.0)
        nc.vector.memset(e0[0:1, :].bitcast(f32), 1.0)
        nc.gpsimd.memset(s_pad[0][:].bitcast(f32), 0.0)
        nc.gpsimd.memset(s_pad[1][:].bitcast(f32), 0.0)

        # PE p-state warmup: ~3us of back-to-back dummy matmuls so the first
        # real projections run at the full 2.4 GHz rate
        warm = pav.tile([128, 512], f32, tag="avy", name="warm")
        for wi_ in range(10):
            nc.tensor.matmul(
                warm[:, 0:128], e64[:], e64[:],
                start=True, stop=True, skip_group_check=True,
            )

        # x is shipped ONLY as bf16 (the cost model's DMA engines serialize
        # all transfers, so halving startup bytes halves the lead-in). wqk
        # leads so the block-0 projections can start immediately.
        xb_r = xb_d.rearrange("(a p) n -> p a n", p=128)
        nc.scalar.dma_start(wqk_s[:], wqk_d.rearrange("(a p) m -> p a m", p=128))
        o, w = NBLKS[0]
        nc.sync.dma_start(xb[:, :, o : o + w], xb_r[:, :, o : o + w])
        nc.scalar.dma_start(wv_s[:], wv_d.rearrange("(a p) m -> p a m", p=128))
        nc.scalar.dma_start(wp_s[:], wp_d)
        for (o, w) in NBLKS[1:]:
            nc.sync.dma_start(xb[:, :, o : o + w], xb_r[:, :, o : o + w])

        # ---------- prologue jobs (per n-block) ----------
        # The per-token norm scale s = sqrt(32/sumsq) is needed in TWO
        # layouts: as a row (-> partition_broadcast -> s_bc, scaling q2/k2
        # along the free axis) and transposed (s_colT, per-partition scalar
        # for the vT scaling). Computing sumsq in both orientations on the
        # PE (ones-vector matmuls) and running the rsqrt bit-trick + Newton
        # on each avoids any DMA transpose round-trip.
        def pro_sumsq(bi):
            o, w = NBLKS[bi]
            c0, cw = o // 128, w // 128

            def job():
                xsq = work.tile([128, KT, 512], bf16, tag="xsq", name=f"xsq_{bi}")
                ps = pav.tile([1, 512], f32, tag="avy", name=f"ps_{bi}")
                for kt in range(KT):
                    nc.vector.tensor_tensor(
                        xsq[:, kt, :w],
                        xb[:, kt, o : o + w],
                        xb[:, kt, o : o + w], OP.mult,
                    )
                    nc.tensor.matmul(
                        ps[:, :w], ones_bf[:, 0:1], xsq[:, kt, :w],
                        start=(kt == 0), stop=(kt == KT - 1),
                    )
                pt = pav.tile([128, 512], f32, tag="avy", name=f"pt_{bi}")
                for tt in range(cw):
                    for kt in range(KT):
                        nc.tensor.matmul(
                            pt[:, tt : tt + 1],
                            xsq[:, kt, tt * 128 : (tt + 1) * 128],
                            ones_bf[:, 0:1],
                            start=(tt == 0 and kt == 0),
                            stop=(tt == cw - 1 and kt == KT - 1),
                            skip_group_check=True,
                        )
                nc.vector.reciprocal(r_rows[:, bi, :w], ps[:, :w])
                nc.vector.reciprocal(r_chunks[:, bi, :cw], pt[:, :cw])
            return job

        def pro_sqrt(bi):
            o, w = NBLKS[bi]
            c0, cw = o // 128, w // 128

            def job():
                # s = sqrt(32 * (1/sumsq)); ACT is idle during the supply
                # phase and all Sqrt instructions precede all Exp in the ACT
                # queue, so only two table loads are charged
                nc.scalar.activation(
                    s_row[:, o : o + w], r_rows[:, bi, :w], AF.Sqrt, scale=32.0
                )
                nc.gpsimd.partition_broadcast(s_bc[:, o : o + w], s_row[:, o : o + w])
                nc.scalar.activation(
                    s_colT[:, c0 : c0 + cw], r_chunks[:, bi, :cw], AF.Sqrt, scale=32.0
                )
            return job

        def pro_k(bi):
            o, w = NBLKS[bi]

            def job():
                pk = pav.tile([128, 512], f32, tag="avy", name=f"pk_{bi}")
                for kt in range(KT):
                    nc.tensor.matmul(
                        pk[:, :w], wqk_s[:, kt, 128:256], xb[:, kt, o : o + w],
                        start=(kt == 0), stop=(kt == KT - 1),
                    )
                nc.vector.tensor_tensor(
                    k2[:, o : o + w], pk[:, :w], s_bc[:, o : o + w], OP.mult
                )
            return job

        def pro_q(bi):
            o, w = NBLKS[bi]

            def job():
                pq = pav.tile([128, 512], f32, tag="avy", name=f"pq_{bi}")
                for kt in range(KT):
                    nc.tensor.matmul(
                        pq[:, :w], wqk_s[:, kt, 0:128], xb[:, kt, o : o + w],
                        start=(kt == 0), stop=(kt == KT - 1),
                    )
                nc.vector.tensor_tensor(
                    q2[:, o : o + w], pq[:, :w], s_bc[:, o : o + w], OP.mult
                )
            return job

        def vt_job(jt):
            def job():
                pv = pav.tile([128, 512], f32, tag="avy", name=f"pv_{jt}")
                for kt in range(KT):
                    nc.tensor.matmul(
                        pv[:, :128], xb[:, kt, jt * 128 : (jt + 1) * 128],
                        wv_s[:, kt, :], start=(kt == 0), stop=(kt == KT - 1),
                    )
                nc.vector.tensor_scalar_mul(
                    vT[:, jt, 0:64], pv[:, 0:64], s_colT[:, jt : jt + 1]
                )
                nc.vector.tensor_scalar_mul(
                    vT[:, jt, 72:136], pv[:, 64:128], s_colT[:, jt : jt + 1]
                )
            return job

        # job queue: prologue for blocks 1.. interleaved with vT jobs, popped
        # during the attention waves (block 0's prologue is emitted eagerly)
        jobs = []
        pro_done = [False] * len(NBLKS)
        vt_done = [False] * JT

        def mark_pro(bi):
            def f():
                pro_done[bi] = True
            return f

        def mark_vt(jt):
            def f():
                vt_done[jt] = True
            return f

        for bi in range(len(NBLKS)):
            pro_sumsq(bi)()
            pro_sqrt(bi)()
            pro_k(bi)()
            pro_q(bi)()
            pro_done[bi] = True
        for jt in range(17, 11, -1):
            vt_job(jt)()
            vt_done[jt] = True

        def late_memsets():
            nc.gpsimd.memset(on[64:65, :, :], 1.0)
            nc.gpsimd.memset(den_pad[0][:].bitcast(f32), 0.0)
            nc.gpsimd.memset(den_pad[1][:].bitcast(f32), 0.0)

        jobs += [(vt_job(jt), mark_vt(jt)) for jt in range(11, -1, -1)]
        jobs.append(late_memsets)

        def pop_job():
            j = jobs.pop(0)
            if isinstance(j, tuple):
                j[0]()
                j[1]()
            else:
                j()

        def ensure_vt(jt):
            while not vt_done[jt]:
                pop_job()

        def ensure_block(bi):
            while not pro_done[bi]:
                pop_job()

        pwav = tc.alloc_tile_pool(name="pwav", bufs=10)
        ywork = tc.alloc_tile_pool(name="ywork", bufs=3)

        # ---------- wave plans ----------
        # full blocks (w=512): alternating G3/G2 tiles of (head, [jts]);
        # 3-waves carry a DR pair + an orphan, 2-waves a DR pair.
        # uniform waves of DR pairs: two psum groups double-buffer (4 banks)
        # leaving FOUR avy banks, so two prologue chains stay in flight and
        # every AV instruction is a DoubleRow pair (no plain-fp8 orphans)
        PAIRS_DESC = [[jt, jt + 1] for jt in range(JT - 2, -1, -2)]

        def wave_plan(w):
            # S slots of width w per psum group (2 banks); waves alternate
            # heads; the odd remainder wave mixes one pair of each head
            S = 1024 // w
            plan = []  # (gtag, size, groups=[(head, jts)])
            a = list(PAIRS_DESC)
            b = list(PAIRS_DESC)
            i = 0
            while a or b:
                if len(a) + len(b) == S // 2 and a and b:
                    groups = []
                    if a:
                        groups.append((0, sum([a.pop(0) for _ in range(len(a))], [])))
                    if b:
                        groups.append((1, sum([b.pop(0) for _ in range(len(b))], [])))
                    size = sum(len(j) for _, j in groups)
                else:
                    head, srcq = (0, a) if i % 2 == 0 else (1, b)
                    npair = min(S // 2, len(srcq))
                    groups = [(head, sum([srcq.pop(0) for _ in range(npair)], []))]
                    size = 2 * npair
                plan.append(("G2A" if i % 2 == 0 else "G2B", size, groups))
                i += 1
            return plan

        def emit_sims(g, o, w, groups):
            slot = 0
            for head, jts in groups:
                for jt in jts:
                    nc.tensor.matmul(
                        g[:, slot, :],
                        k2[64 * head : 64 * (head + 1), jt * 128 : (jt + 1) * 128],
                        q2[64 * head : 64 * (head + 1), o : o + w],
                        start=True, stop=True,
                    )
                    slot += 1

        def emit_avs(psb, w, groups, av, avflags):
            slot = 0
            for head, jts in groups:
                vbase = 72 * head
                flags = avflags[head]
                for i in range(0, len(jts), 2):
                    assert jts[i + 1] == jts[i] + 1
                    nc.tensor.matmul(
                        av[head][:, :w],
                        vT[:, jts[i] : jts[i] + 2, vbase : vbase + 65],
                        psb[:, slot : slot + 2, :],
                        start=flags["first"], stop=flags["remaining"] == 2,
                        perf_mode=DRM, skip_group_check=True,
                    )
                    slot += 2
                    flags["remaining"] -= 2
                    flags["first"] = False

        def make_tail_norm(ib, o, w, av, h):
            def tail():
                with nc.allow_low_precision(reason="1/den broadcast via f32r matmul"):
                    nc.vector.reciprocal(den_pad[h][64:65, :w], av[h][64:65, :w])
                dbc = pav.tile([128, 512], f32, tag="avy", name=f"dbc_{ib}_{h}")
                nc.tensor.matmul(
                    dbc[:, :w], e64[:], den_pad[h][:, :w], start=True, stop=True
                )
                rb = work.tile([128, 512], f32, tag="rb", name=f"rb_{ib}_{h}")
                nc.vector.tensor_copy(rb[:, :w], dbc[:, :w])
                nc.vector.tensor_tensor(
                    on[0:64, h, o : o + w], av[h][0:64, :w], rb[0:64, :w], OP.mult
                )
            return tail

        def make_tail_proj(ib, o, w):
            def tail():
                ysb = ywork.tile([128, KT, 512], f32, tag="y", name=f"ysb_{ib}")
                y_r = y_d.rearrange("(a p) n -> p a n", p=128)
                for ot in range(KT):
                    py = pav.tile([128, 512], f32, tag="avy", name=f"py_{ib}_{ot}")
                    nc.tensor.matmul(
                        py[:, :w], wp_s[:, :, ot * 128 : (ot + 1) * 128],
                        on[:, :, o : o + w],
                        start=True, stop=True, perf_mode=DRM,
                    )
                    nc.vector.tensor_tensor(
                        ysb[:, ot, :w], py[:, :w], xb[:, ot, o : o + w], OP.add
                    )
                    if ot == 1:
                        nc.sync.dma_start(y_r[:, 0:2, o : o + w], ysb[:, 0:2, :w])
                nc.sync.dma_start(y_r[:, 2:4, o : o + w], ysb[:, 2:4, :w])
            return tail

        # ---------- attention ----------
        deferred = []
        for ib, (o, w) in enumerate(IBLKS):
            ensure_block(BLK_OF_COL[o])
            plan = wave_plan(w)
            av = [
                pav.tile([65, 512], f32, tag="avy", name=f"av0_{ib}"),
                pav.tile([65, 512], f32, tag="avy", name=f"av1_{ib}"),
            ]
            avflags = [{"first": True, "remaining": JT}, {"first": True, "remaining": JT}]
            pending = None
            for wv_i, (gtag, size, groups) in enumerate(plan):
                g = pg.tile([128, size, w], f32, tag=gtag, name=f"g_{ib}_{wv_i}")
                emit_sims(g, o, w, groups)
                p_sb = pwav.tile([128, size, w], f8, tag="P", name=f"p_{ib}_{wv_i}")
                nc.scalar.activation(p_sb[:], g[:], AF.Exp, bias=bias_m2[:])
                if deferred and wv_i == 0:
                    deferred.pop(0)()
                waves = [pending, (p_sb, groups)] if pending else [(p_sb, groups)]
                if wv_i < len(plan) - 1:
                    pending = waves.pop()
                for psb_j, groups_j in waves:
                    for _, jts_j in groups_j:
                        for jt in jts_j:
                            ensure_vt(jt)
                    for _ in range(4):
                        if jobs:
                            pop_job()
                    emit_avs(psb_j, w, groups_j, av, avflags)
            make_tail_norm(ib, o, w, av, 0)()
            make_tail_norm(ib, o, w, av, 1)()
            deferred = [make_tail_proj(ib, o, w)]
        while jobs:
            pop_job()
        for fn in deferred:
            fn()

        if debug:
            nc.sync.dma_start(dbg["q2"], q2[:].bitcast(f32))
            nc.sync.dma_start(dbg["k2"], k2[:].bitcast(f32))
            vtf = big.tile([128, JT, 144], f32)
            nc.vector.tensor_copy(vtf[:], vT[:])
            nc.sync.dma_start(dbg["vT"], vtf[:])
            onf = big.tile([65, 2, N], f32)
            nc.vector.tensor_copy(onf[:], on[:])
            nc.sync.dma_start(dbg["on"], onf[:])
            nc.sync.dma_start(dbg["s_bc"], s_bc[:])
            nc.sync.dma_start(dbg["s_colT"], s_colT[:])
        for pool in (ywork, pwav, pav, pg, rsq, work, big):
            pool.release()

    nc.compile()
    return nc


def _get_program():
    if "nc" not in _CACHE:
        _CACHE["nc"] = _build_program()
    return _CACHE["nc"]


def make_in_maps(x, g, w_qkv, w_out, b_out):
    """Build the per-core input dicts for the SPMD launch."""
    import ml_dtypes

    x = np.asarray(x, dtype=np.float32)
    g = np.asarray(g, dtype=np.float32).reshape(DIM)
    w_qkv = np.asarray(w_qkv, dtype=np.float32)
    w_out = np.asarray(w_out, dtype=np.float32)
    b_out = np.asarray(b_out, dtype=np.float32)

    in_maps = []
    for c in range(8):
        beta = c // 4
        h0 = 2 * (c % 4)
        h1 = h0 + 1
        x4 = (x[beta].reshape(DIM, N) / 4.0).astype(np.float32)
        # w_qkv rows: q block [0:512], k block [512:1024], v block [1024:1536]
        qr = np.r_[h0 * DH : (h0 + 1) * DH, h1 * DH : (h1 + 1) * DH]
        wq = w_qkv[qr]            # [128, DIM]
        wk = w_qkv[DIM + qr]      # [128, DIM]
        wvv = w_qkv[2 * DIM + qr]  # [128, DIM]
        gw = (g[None, :] * 4.0).astype(np.float32)
        # fold the attention 1/8 scale into wq so q2 and k2 share s_bc
        wqk = np.concatenate([wq * gw / 8.0, wk * gw], axis=0).T.copy()  # [DIM, 256]
        wvt = (wvv * gw).T.astype(ml_dtypes.bfloat16)  # [DIM, 128]
        wp = np.zeros((65, 2, DIM), dtype=np.float32)
        wp[0:64, 0, :] = w_out[:, h0 * DH : (h0 + 1) * DH].T
        wp[0:64, 1, :] = w_out[:, h1 * DH : (h1 + 1) * DH].T
        wp[64, :, :] = b_out[None, :] / 8.0
        in_maps.append(
            {
                "xbin": np.ascontiguousarray(x4.astype(ml_dtypes.bfloat16)),
                "wqk": np.ascontiguousarray(wqk.astype(ml_dtypes.bfloat16)),
                "wv": np.ascontiguousarray(wvt),
                "wp": wp.astype(ml_dtypes.float8_e4m3),
            }
        )
    return in_maps


def run_spmd(in_maps, trace=False):
    from concourse.bass_utils import run_bass_kernel_spmd

    nc = _get_program()
    return run_bass_kernel_spmd(nc, in_maps, list(range(8)), trace=trace)


def combine(results, x):
    x = np.asarray(x, dtype=np.float32)
    y = np.zeros((B, DIM, N), dtype=np.float32)
    for c in range(8):
        y[c // 4] += results[c]["y"]
    return y.reshape(B, DIM, HWS, HWS)


def kernel(x, g, w_qkv, w_out, b_out):
    in_maps = make_in_maps(x, g, w_qkv, w_out, b_out)
    res = run_spmd(in_maps)
    return combine(res.results, x)
